# revision 1
# baseline (speedup 1.0000x reference)
"""GraphUnpooling Trainium2 kernel.

out[b, j, f, h] = x[b, fine_to_coarse[j], f, h]
x: [4, 2000, 4, 64] f32, fine_to_coarse: [50000] i32 -> out: [4, 50000, 4, 64] f32

Sharding: 8 cores = 4 batches x 2 fine-halves. Each core produces 25000
output rows (padded to 25600) of one batch, gathering from that batch's
2000-row coarse table.

Two gather engines, selectable via MODE:
  - "dma": gpsimd.dma_gather HBM->SBUF (1KB/row descriptors), then
    contiguous DMA write-out.  All traffic on the SDMA engines.
  - "apg": coarse table lives in SBUF split across partitions
    (two [128, 2000] tables = fh 0:128 / 128:256, host-pretransposed);
    gpsimd.ap_gather does the gather on-chip; SDMA only writes output.
  - "hybrid": first NA rows via "dma", rest via "apg" (single library
    transition).
"""

import numpy as np

import concourse.bacc as bacc
import concourse.mybir as mybir
import concourse.tile as tile
from concourse.bass_utils import run_bass_kernel_spmd

B, NCOARSE, F, H = 4, 2000, 4, 64
E = F * H  # 256
NF = 50000
HALF = NF // 2  # 25000 rows per core
NP = 25600  # padded rows per core
N_CORES = 8

CONFIG = {
    "mode": "dma",   # dma_gather (HBM->SBUF, 1KB/row) + staged 2.56MB write-out
    "CHA": 5120,     # rows per write chunk (5 chunks; each fed by 5 dma_gathers)
    "CHB": 3200,     # rows per ap_gather chunk (apg/hybrid modes only)
    "NA": 6400,      # hybrid: rows on the dma path (multiple of CHA)
    "bufs_a": 2,
    "bufs_b": 3,
    "REP": 1,        # repeat the pass in a For_i loop (benchmarking only)
}

_PROG_CACHE = {}


def _build_program(mode, CHA, CHB, NA, bufs_a, bufs_b, REP=1):
    f32 = mybir.dt.float32
    i16 = mybir.dt.int16

    if mode == "dma":
        NA, NB = NP, 0
    elif mode == "apg":
        NA, NB = 0, NP
    else:
        NB = NP - NA
    assert NA % CHA == 0 if NA else True
    assert CHA % 128 == 0
    assert NB % CHB == 0 if NB else True

    nc = bacc.Bacc("TRN2", target_bir_lowering=False, debug=False)

    idxw = nc.dram_tensor("idxw", [128, NP // 16], i16, kind="ExternalInput")
    if NA:
        xb = nc.dram_tensor("xb", [NCOARSE, E], f32, kind="ExternalInput")
        outa = nc.dram_tensor("outa", [128, NA // 128, E], f32, kind="ExternalOutput")
    if NB:
        xbt = nc.dram_tensor("xbt", [E, NCOARSE], f32, kind="ExternalInput")
        outb = nc.dram_tensor("outb", [E, NB], f32, kind="ExternalOutput")

    with tile.TileContext(nc) as tc:
        with (
            tc.tile_pool(name="const", bufs=1) as cpool,
            tc.tile_pool(name="pa", bufs=bufs_a) as pa,
            tc.tile_pool(name="pb", bufs=bufs_b) as pb,
        ):
            idx_sb = cpool.tile([128, NP // 16], i16, tag="idx")
            nc.sync.dma_start(out=idx_sb[:], in_=idxw[:])
            if NB:
                x0 = cpool.tile([128, NCOARSE], f32, tag="x0")
                x1 = cpool.tile([128, NCOARSE], f32, tag="x1")
                nc.sync.dma_start(out=x0[:], in_=xbt[0:128, :])
                nc.sync.dma_start(out=x1[:], in_=xbt[128:256, :])

            GCH = 1024  # dma_gather chunk (single_packet ring limit)

            def one_pass():
                # --- dma_gather phase (library: mlp) ---
                for k in range(NA // CHA if NA else 0):
                    j0 = k * CHA
                    ga = pa.tile([128, CHA // 128, E], f32, tag="ga")
                    off = 0
                    while off < CHA:
                        g_sz = min(GCH, CHA - off)
                        jg = j0 + off
                        nc.gpsimd.dma_gather(
                            out_ap=ga[:, off // 128 : (off + g_sz) // 128, :],
                            in_ap=xb[:],
                            idxs_ap=idx_sb[:, jg // 16 : (jg + g_sz) // 16],
                            num_idxs=g_sz,
                            num_idxs_reg=g_sz,
                            elem_size=E,
                        )
                        off += g_sz
                    c0 = j0 // 128
                    nc.sync.dma_start(
                        out=outa[:, c0 : c0 + CHA // 128, :], in_=ga[:]
                    )

                # --- ap_gather phase (library: ap_gather) ---
                for k in range(NB // CHB if NB else 0):
                    j0 = k * CHB
                    o0 = pb.tile([128, CHB], f32, tag="o0")
                    o1 = pb.tile([128, CHB], f32, tag="o1")
                    idxs = idx_sb[:, (NA + j0) // 16 : (NA + j0 + CHB) // 16]
                    nc.gpsimd.ap_gather(
                        out_ap=o0[:], in_ap=x0[:], idxs_ap=idxs,
                        channels=128, num_elems=NCOARSE, d=1, num_idxs=CHB,
                    )
                    nc.gpsimd.ap_gather(
                        out_ap=o1[:], in_ap=x1[:], idxs_ap=idxs,
                        channels=128, num_elems=NCOARSE, d=1, num_idxs=CHB,
                    )
                    nc.sync.dma_start(out=outb[0:128, j0 : j0 + CHB], in_=o0[:])
                    nc.sync.dma_start(out=outb[128:256, j0 : j0 + CHB], in_=o1[:])

            if REP > 1:
                with tc.For_i(0, REP, 1):
                    one_pass()
            else:
                one_pass()
    nc.compile()
    return nc, NA, NB


def _get_program():
    key = (
        CONFIG["mode"], CONFIG["CHA"], CONFIG["CHB"], CONFIG["NA"],
        CONFIG["bufs_a"], CONFIG["bufs_b"], CONFIG.get("REP", 1),
    )
    if key not in _PROG_CACHE:
        _PROG_CACHE[key] = _build_program(*key)
    return _PROG_CACHE[key]


def _wrap_idx(idx_half):
    """[25000] i32 -> [128, 1600] i16 wrap-16 layout replicated to 8 groups."""
    pad = np.zeros(NP, dtype=np.int16)
    pad[:HALF] = idx_half.astype(np.int16)
    w = pad.reshape(NP // 16, 16).T  # [16, NP/16]
    return np.ascontiguousarray(np.tile(w, (8, 1)))


def kernel(x, fine_to_coarse, _trace=False, _trace_kwargs=None):
    x = np.ascontiguousarray(np.asarray(x, dtype=np.float32))
    idx = np.asarray(fine_to_coarse, dtype=np.int32)

    nc, NA, NB = _get_program()

    in_maps = []
    for c in range(N_CORES):
        b, h = divmod(c, 2)
        m = {"idxw": _wrap_idx(idx[h * HALF : (h + 1) * HALF])}
        xflat = x[b].reshape(NCOARSE, E)
        if NA:
            m["xb"] = xflat
        if NB:
            m["xbt"] = np.ascontiguousarray(xflat.T)
        in_maps.append(m)

    res = run_bass_kernel_spmd(
        nc, in_maps, list(range(N_CORES)),
        trace=_trace, **(_trace_kwargs or {}),
    )

    out = np.empty((B, NF, F, H), dtype=np.float32)
    for c in range(N_CORES):
        b, h = divmod(c, 2)
        r = res.results[c]
        parts = []
        if NA:
            parts.append(r["outa"].transpose(1, 0, 2).reshape(NA, E))
        if NB:
            parts.append(np.ascontiguousarray(r["outb"].T))
        rows = parts[0] if len(parts) == 1 else np.concatenate(parts, axis=0)
        out[b, h * HALF : (h + 1) * HALF] = rows[:HALF].reshape(HALF, F, H)
    if _trace:
        kernel._last_result = res
    return out

